# revision 5
# baseline (speedup 1.0000x reference)
"""Distributed CL loss kernel for Trainium2 (8 NeuronCores).

Reference computes  mean_i sum_j ||s_i - t_j||^2 * [tg_i == tg_j] / cnt[tg_i]
with the [N, N] pairwise-distance matrix.  Because the mask only depends on
the class labels, the whole loss collapses to per-class aggregates:

  sum_j d2[i,j]*mask[i,j] = cnt[c_i]*|s_i|^2 + sum_{j in c_i}|t_j|^2
                            - 2 * s_i . T_{c_i}
  loss = (1/N) * [ sum_i |s_i|^2 + sum_j |t_j|^2 - 2 * sum_c S_c.T_c / cnt_c ]

with S_c / T_c the class-sums of fm_s / fm_t rows.  So the device work is a
single streaming pass: class-sum matmuls (one-hot^T @ X on the PE, fp32r
single-pass) plus sum-of-squares reductions (fm_s on ACT via Square+accum,
fm_t on DVE via mul+reduce), sharded by rows across the 8 cores.  The
remaining O(C*D) combine runs on the host while gathering.

fp32r notes: matmul inputs are declared float32r (same f32 bits; the PE
streams them single-pass at ~TF32 effective precision, 4x faster than the
fp32 two-pass path).  That reduced precision only touches the class-sum
cross term, which contributes ~0.1% of the loss magnitude — measured final
relative error stays ~1e-6.  The sum-of-squares paths read the same SBUF
bytes bitcast back to plain f32, so the dominant |s|^2+|t|^2 terms keep
full fp32 precision.
"""

import numpy as np

N, D, NUM_CLASSES = 4096, 1024, 10
NCORES = 8
RPC = N // NCORES  # rows per core (both fm_s and fm_t are row-sharded)
KT = RPC // 128    # 128-row k-tiles per core
CP = 16            # class dim padded for alignment

_STATE = {}
LAST_RUN = None  # BassKernelResults of the most recent device run (for test.py)


def _build_nc_tile():
    import concourse.bacc as bacc
    import concourse.mybir as mybir
    import concourse.tile as tile

    f32 = mybir.dt.float32
    f32r = mybir.dt.float32r
    nc = bacc.Bacc(
        "TRN2",
        target_bir_lowering=False,
        debug=False,
        enable_asserts=False,
        num_devices=NCORES,
    )

    s_in = nc.dram_tensor("s_in", (RPC, D), f32r, kind="ExternalInput")
    t_in = nc.dram_tensor("t_in", (RPC, D), f32r, kind="ExternalInput")
    oh_in = nc.dram_tensor("oh_in", (RPC, CP), f32r, kind="ExternalInput")
    S_out = nc.dram_tensor("S_out", (CP, D), f32, kind="ExternalOutput")
    T_out = nc.dram_tensor("T_out", (CP, D), f32, kind="ExternalOutput")
    st_out = nc.dram_tensor("st_out", (128, 2 * KT), f32, kind="ExternalOutput")

    # row r = n*128 + p  ->  partition p, k-tile n
    s_r = s_in.ap().rearrange("(n p) d -> p n d", p=128)
    t_r = t_in.ap().rearrange("(n p) d -> p n d", p=128)
    oh_r = oh_in.ap().rearrange("(n p) c -> p n c", p=128)

    with tile.TileContext(nc) as tc:
        with (
            tc.tile_pool(name="data", bufs=KT) as data_pool,
            tc.tile_pool(name="scratch", bufs=2) as scratch_pool,
            tc.tile_pool(name="small", bufs=1) as small_pool,
            tc.tile_pool(name="psum", bufs=1, space="PSUM") as psum_pool,
        ):
            oh_sb = small_pool.tile([128, KT, CP], f32r, tag="oh")
            nc.sync.dma_start(oh_sb[:], oh_r)
            stats = small_pool.tile([128, 2 * KT], f32, tag="stats")

            pS0 = psum_pool.tile([CP, 512], f32, tag="pS0")
            pS1 = psum_pool.tile([CP, 512], f32, tag="pS1")
            pT0 = psum_pool.tile([CP, 512], f32, tag="pT0")
            pT1 = psum_pool.tile([CP, 512], f32, tag="pT1")

            for k in range(KT):
                start, stop = k == 0, k == KT - 1
                s_t = data_pool.tile([128, D], f32r, tag="s")
                nc.sync.dma_start(s_t[:], s_r[:, k, :])
                t_t = data_pool.tile([128, D], f32r, tag="t")
                nc.gpsimd.dma_start(t_t[:], t_r[:, k, :])
                oh_k = oh_sb[:, k, :]

                nc.tensor.matmul(pS0[:], oh_k, s_t[:, 0:512], start=start, stop=stop)
                nc.tensor.matmul(pS1[:], oh_k, s_t[:, 512:D], start=start, stop=stop)
                nc.tensor.matmul(pT0[:], oh_k, t_t[:, 0:512], start=start, stop=stop)
                nc.tensor.matmul(pT1[:], oh_k, t_t[:, 512:D], start=start, stop=stop)

                # |s|^2 on ACT: fused square + free-axis accumulate
                sq_s = scratch_pool.tile([128, D], f32, tag="sq_s")
                nc.scalar.activation(
                    sq_s[:],
                    s_t[:].bitcast(f32),
                    mybir.ActivationFunctionType.Square,
                    accum_out=stats[:, k : k + 1],
                )
                # |t|^2 on DVE: square then reduce (tensor_tensor_reduce
                # mis-executes on HW, so two plain ops)
                sq_t = scratch_pool.tile([128, D], f32, tag="sq_t")
                nc.vector.tensor_mul(
                    sq_t[:], t_t[:].bitcast(f32), t_t[:].bitcast(f32)
                )
                nc.vector.reduce_sum(
                    stats[:, KT + k : KT + k + 1],
                    sq_t[:],
                    axis=mybir.AxisListType.X,
                )

            S_sb = small_pool.tile([CP, D], f32, tag="S_sb")
            T_sb = small_pool.tile([CP, D], f32, tag="T_sb")
            nc.scalar.copy(S_sb[:, 0:512], pS0[:])
            nc.scalar.copy(S_sb[:, 512:D], pS1[:])
            nc.vector.tensor_copy(T_sb[:, 0:512], pT0[:])
            nc.vector.tensor_copy(T_sb[:, 512:D], pT1[:])

            nc.sync.dma_start(S_out.ap(), S_sb[:])
            nc.sync.dma_start(T_out.ap(), T_sb[:])
            nc.sync.dma_start(st_out.ap(), stats[:])

    nc.compile()
    return nc


def build_nc_raw():
    import concourse.bacc as bacc
    import concourse.mybir as mybir

    f32 = mybir.dt.float32
    f32r = mybir.dt.float32r
    nc = bacc.Bacc(
        "TRN2",
        target_bir_lowering=False,
        debug=False,
        enable_asserts=False,
        num_devices=NCORES,
    )

    s_in = nc.dram_tensor("s_in", (RPC, D), f32r, kind="ExternalInput")
    t_in = nc.dram_tensor("t_in", (RPC, D), f32r, kind="ExternalInput")
    oh_in = nc.dram_tensor("oh_in", (RPC, CP), f32r, kind="ExternalInput")
    ST_out = nc.dram_tensor("ST_out", (CP, 2 * D), f32, kind="ExternalOutput")
    st_out = nc.dram_tensor("st_out", (128, 2 * KT), f32, kind="ExternalOutput")

    s_r = s_in.ap().rearrange("(n p) d -> p n d", p=128)
    t_r = t_in.ap().rearrange("(n p) d -> p n d", p=128)
    oh_r = oh_in.ap().rearrange("(n p) c -> p n c", p=128)

    s_sb = nc.alloc_sbuf_tensor("s_sb", [128, KT, D], f32r)
    t_sb = nc.alloc_sbuf_tensor("t_sb", [128, KT, D], f32r)
    oh_sb = nc.alloc_sbuf_tensor("oh_sb", [128, KT, CP], f32r)
    sq_s = nc.alloc_sbuf_tensor("sq_s", [128, 2, D], f32)
    sq_t = nc.alloc_sbuf_tensor("sq_t", [128, 2, D], f32)
    stats = nc.alloc_sbuf_tensor("stats", [128, 2 * KT], f32)
    out_sb = nc.alloc_sbuf_tensor("out_sb", [CP, 2 * D], f32)

    pS0 = nc.alloc_psum_tensor("pS0", [CP, 512], f32)
    pS1 = nc.alloc_psum_tensor("pS1", [CP, 512], f32)
    pT0 = nc.alloc_psum_tensor("pT0", [CP, 512], f32)
    pT1 = nc.alloc_psum_tensor("pT1", [CP, 512], f32)

    oh_sem = nc.alloc_semaphore("oh_sem")
    s_sems = [nc.alloc_semaphore(f"s_sem{k}") for k in range(KT)]
    t_sems = [nc.alloc_semaphore(f"t_sem{k}") for k in range(KT)]
    pS_done = nc.alloc_semaphore("pS_done")
    pT_done = nc.alloc_semaphore("pT_done")
    act_done = nc.alloc_semaphore("act_done")
    dve_done = nc.alloc_semaphore("dve_done")
    dve_mul = nc.alloc_semaphore("dve_mul")
    copy_done = nc.alloc_semaphore("copy_done")
    out_sem = nc.alloc_semaphore("out_sem")

    Sq = mybir.ActivationFunctionType.Square
    X = mybir.AxisListType.X

    with nc.Block() as block:

        @block.sync
        def _(sync):
            # ACT issues the k=0 tile loads in parallel (both are HWDGE
            # engines); SP covers the one-hot + the rest, s3 last.
            sync.dma_start(oh_sb[:], oh_r).then_inc(oh_sem, 16)
            for k in range(1, KT - 1):
                sync.dma_start(s_sb[:, k, :], s_r[:, k, :]).then_inc(s_sems[k], 16)
                sync.dma_start(t_sb[:, k, :], t_r[:, k, :]).then_inc(t_sems[k], 16)
            k = KT - 1
            sync.dma_start(t_sb[:, k, :], t_r[:, k, :]).then_inc(t_sems[k], 16)
            sync.dma_start(s_sb[:, k, :], s_r[:, k, :]).then_inc(s_sems[k], 16)

            sync.wait_ge(act_done, KT)
            sync.wait_ge(dve_done, KT)
            sync.dma_start(st_out.ap(), stats[:]).then_inc(out_sem, 16)
            sync.wait_ge(out_sem, 32)

        @block.tensor
        def _(tensor):
            tensor.wait_ge(oh_sem, 16)
            for k in range(KT):
                start, stop = k == 0, k == KT - 1
                oh_k = oh_sb[:, k, :]
                tensor.wait_ge(t_sems[k], 16)
                tensor.matmul(pT0[:], oh_k, t_sb[:, k, 0:512], start=start, stop=stop)
                mmt = tensor.matmul(
                    pT1[:], oh_k, t_sb[:, k, 512:D], start=start, stop=stop
                )
                if stop:
                    mmt.then_inc(pT_done, 1)
                tensor.wait_ge(s_sems[k], 16)
                tensor.matmul(pS0[:], oh_k, s_sb[:, k, 0:512], start=start, stop=stop)
                mms = tensor.matmul(
                    pS1[:], oh_k, s_sb[:, k, 512:D], start=start, stop=stop
                )
                if stop:
                    mms.then_inc(pS_done, 1)

        @block.scalar
        def _(scalar):
            scalar.dma_start(s_sb[:, 0, :], s_r[:, 0, :]).then_inc(s_sems[0], 16)
            scalar.dma_start(t_sb[:, 0, :], t_r[:, 0, :]).then_inc(t_sems[0], 16)
            for k in range(KT):
                scalar.wait_ge(s_sems[k], 16)
                if k >= 2:
                    # scratch buffer k%2 free once square k-2 fully retired
                    scalar.wait_ge(act_done, k - 1)
                scalar.activation(
                    sq_s[:, k % 2, :],
                    s_sb[:, k, :].bitcast(f32),
                    Sq,
                    accum_out=stats[:, k : k + 1],
                ).then_inc(act_done, 1)
            scalar.wait_ge(pS_done, 1)
            scalar.copy(out_sb[:, 0:512], pS0[:])
            scalar.copy(out_sb[:, 512:D], pS1[:]).then_inc(copy_done, 1)
            scalar.wait_ge(copy_done, 2)
            scalar.dma_start(ST_out.ap(), out_sb[:]).then_inc(out_sem, 16)

        @block.vector
        def _(vector):
            for k in range(KT):
                vector.wait_ge(t_sems[k], 16)
                if k >= 2:
                    vector.wait_ge(dve_done, k - 1)
                vector.tensor_mul(
                    sq_t[:, k % 2, :], t_sb[:, k, :].bitcast(f32), t_sb[:, k, :].bitcast(f32)
                ).then_inc(dve_mul, 1)
                vector.wait_ge(dve_mul, k + 1)
                vector.reduce_sum(
                    stats[:, KT + k : KT + k + 1], sq_t[:, k % 2, :], axis=X
                ).then_inc(dve_done, 1)
            vector.wait_ge(pT_done, 1)
            vector.tensor_copy(out_sb[:, D : D + 512], pT0[:])
            vector.tensor_copy(out_sb[:, D + 512 : 2 * D], pT1[:]).then_inc(
                copy_done, 1
            )

    nc.compile()
    return nc


def _build_nc():
    import os
    if os.environ.get("KERNEL_IMPL", "raw") == "tile":
        return _build_nc_tile()
    return build_nc_raw()


def _get_nc():
    if "nc" not in _STATE:
        _STATE["nc"] = _build_nc()
    return _STATE["nc"]


def kernel(fm_s, fm_t, targets, fusion_true=0, **_unused):
    global LAST_RUN
    from concourse.bass_utils import run_bass_kernel_spmd

    fm_s = np.ascontiguousarray(np.asarray(fm_s, dtype=np.float32))
    fm_t = np.ascontiguousarray(np.asarray(fm_t, dtype=np.float32))
    tg = np.asarray(targets).astype(np.int64).ravel()
    assert fm_s.shape == (N, D) and fm_t.shape == (N, D) and tg.shape == (N,)

    oh = (tg[:, None] == np.arange(CP, dtype=np.int64)[None, :]).astype(np.float32)
    counts = np.bincount(tg, minlength=CP).astype(np.float64)[:CP]

    in_maps = [
        {
            "s_in": fm_s[c * RPC : (c + 1) * RPC],
            "t_in": fm_t[c * RPC : (c + 1) * RPC],
            "oh_in": oh[c * RPC : (c + 1) * RPC],
        }
        for c in range(NCORES)
    ]

    nc = _get_nc()
    LAST_RUN = run_bass_kernel_spmd(nc, in_maps, list(range(NCORES)))
    res = LAST_RUN.results

    S = np.zeros((CP, D), np.float64)
    T = np.zeros((CP, D), np.float64)
    ss = 0.0
    tt = 0.0
    for r in res:
        if "ST_out" in r:
            S += r["ST_out"][:, :D].astype(np.float64)
            T += r["ST_out"][:, D:].astype(np.float64)
        else:
            S += r["S_out"].astype(np.float64)
            T += r["T_out"].astype(np.float64)
        ss += float(r["st_out"][:, :KT].astype(np.float64).sum())
        tt += float(r["st_out"][:, KT:].astype(np.float64).sum())

    safe = np.where(counts > 0, counts, 1.0)
    dot = float(((S * T).sum(axis=1) / safe).sum())
    loss = (ss + tt - 2.0 * dot) / N
    return np.array(loss, dtype=np.float32)
